# revision 43
# baseline (speedup 1.0000x reference)
"""Trainium2 Bass kernel for multi-head self-attention (dense transformer block).

Reference computation (per batch element b):
    qkv  = x @ w_in.T + b_in                      # [N, 3C]
    q,k,v per head (H=16, D=64)
    S    = (q @ k.T) * D**-0.5
    P    = softmax(S)                             # rows over keys
    attn = P @ v                                  # [N, C] after head merge
    y    = attn @ w_out.T + b_out

Sharding: data-parallel over batch. B=8 maps one batch element per NeuronCore.

Per-core dataflow (bf16 matmuls, fp32 PSUM accumulation; the softmax
normalization chain runs in fp32r for accuracy):
  - host pre-transposes x[b] -> xT [C, N] and the weights (w_inT [C,3C],
    w_outT [C,C]) and casts them to bf16, so every matmul operand loads with
    a contiguous DMA at full PE rate (1 elem/cycle, FWL weight loads).
  - in_proj produces qT,kT in feature-major layout [f, n] (f on partitions,
    bias fused into the PSUM eviction as a per-partition DVE add) and v in
    token-major layout [n, f] (bias via a K=1 ones-matmul into the PSUM
    accumulation).  v is stored per head with a 65th all-ones column.
  - attention computes S^T per head ([keys, queries] layout) so that
    P^T = exp(S^T * scale) comes out with keys on partitions, ready to be the
    moving operand of the P@V matmul (contraction over keys).  Softmax skips
    the max-subtraction (scores are O(5) so exp is safe in fp32), which lets
    exp fuse into the PSUM->SBUF eviction on the scalar engine as one
    [128, 1024] op per (head, key-tile).  The P@V matmul uses the [v | ones]
    stationary (M=65): row 64 of the output is the softmax denominator,
    computed for free on the tensor engine.  Because the denominator sums the
    same bf16-rounded P used by P@V, the normalization is exact with respect
    to the rounding.
  - normalization: 1/rowsum on DVE (fp32r), broadcast across partitions via a
    K=1 fp32r ones-matmul (fp32r matmuls must write PSUM at partition 0, so
    head 1's normalized tile takes an SBUF->SBUF DMA hop to partitions
    64-127).
  - out_proj consumes the head-merged attnT [c, n] directly as the stationary
    operand; bias again via a K=1 ones-matmul.
  - the q/k projection matmuls of head pair hp+1 are interleaved into the
    attention of pair hp so the tensor engine stays saturated while the
    scalar engine drains the exp evictions (keeps the HAM clock gate at 8/8).

Optimizations over the original pipeline (308us -> ~266us):
  - PE warmup spin at kernel start (memset-built ones, no DMA dependency)
    so the HAM clock gate is at 8/8 before the v projection streams.
  - score psum double-buffered as 2x[128,1024] tiles: breaks the
    exp(k) -> scores(k+1) serialization that clocked the old rounds at
    ~2.9us/step; rounds now run at the ACTIVATE floor (~2.2us/step).
  - per key-tile step the round interleaves scores, P@V of the previous
    pair and a q/k chunk of the next pair, so the PE tracks the exp floor.
  - the second v-projection slab (heads 8-15, first needed in round 5) is
    deferred into round 0's empty P@V slot, hiding ~14us of front time.
  - softmax normalization split in two phases: the psum-freeing copies +
    reciprocal spread/broadcast DMAs launch at P@V stop, but the final
    multiplies are emitted ~2 steps later when their broadcasts have
    landed — the vector engine is strict FIFO, and a waiting mult would
    head-of-line block the copies that recycle the P@V psum banks.
  - split-y out projection: all waves accumulate head pairs 0..6 and bank
    the partial in SBUF (bf16); the pair-7 k-steps trail 3 waves behind,
    giving pair 7's normalization chains a ~15us runway before anything
    waits on them.
"""
import numpy as np
from contextlib import ExitStack

import concourse.bass as bass  # noqa: F401
from concourse import bacc
import concourse.tile as tile
from concourse import mybir

F32 = mybir.dt.float32
F32R = mybir.dt.float32r
BF16 = mybir.dt.bfloat16
EXP = mybir.ActivationFunctionType.Exp

B = 8
N = 1024          # tokens
C = 1024          # hidden
H = 16            # heads
D = C // H        # 64
F3 = 3 * C
SCALE = float(D) ** -0.5
P = 128
CT = C // P       # 8 contraction tiles over C
NT = N // P       # 8 token tiles
HP = H // 2       # 8 head pairs (two heads share a 128-partition tile)
IB = 512          # query-block (matmul moving free dim)
NIB = N // IB     # 2

_CACHE = {}
LAST_EXEC_TIME_NS = None


def _build():
    nc = bacc.Bacc("TRN2", target_bir_lowering=False, debug=False)
    xTt = nc.dram_tensor("xTt", [P, CT * N], BF16, kind="ExternalInput")
    wqk = nc.dram_tensor("wqk", [HP * 2 * P, CT * P], BF16,
                         kind="ExternalInput")
    wv = nc.dram_tensor("wv", [2 * P, CT * IB], BF16, kind="ExternalInput")
    wo = nc.dram_tensor("wo", [2 * P, CT * IB], BF16, kind="ExternalInput")
    b_qk_pm = nc.dram_tensor("b_qk_pm", [P, 2 * CT], F32, kind="ExternalInput")
    b_v_bf = nc.dram_tensor("b_v_bf", [C], BF16, kind="ExternalInput")
    b_o_bf = nc.dram_tensor("b_o_bf", [C], BF16, kind="ExternalInput")
    y = nc.dram_tensor("y", [N, C], F32, kind="ExternalOutput")

    with tile.TileContext(nc) as tc:
        with ExitStack() as ctx:
            consts = ctx.enter_context(tc.tile_pool(name="consts", bufs=1))
            qkp = ctx.enter_context(tc.tile_pool(name="qk", bufs=7))
            vp = ctx.enter_context(tc.tile_pool(name="v", bufs=1))
            atp = ctx.enter_context(tc.tile_pool(name="attnT", bufs=1))
            wbp = ctx.enter_context(tc.tile_pool(name="wB", bufs=16))
            yp = ctx.enter_context(tc.tile_pool(name="y", bufs=3))
            r2p = ctx.enter_context(tc.tile_pool(name="r2", bufs=4))
            pvcp = ctx.enter_context(tc.tile_pool(name="pvc", bufs=6))
            rbp = ctx.enter_context(tc.tile_pool(name="rb", bufs=6))
            rdp = ctx.enter_context(tc.tile_pool(name="rd", bufs=16, space="DRAM"))

            # ---- persistent big tensors ----
            # q/k in feature-major layout, one tile per (region, head pair) so
            # attention of pair hp depends only on its own projection tiles.
            qk_t = {}                           # allocated lazily, 8 slots rotate
            v_ext = vp.tile([P, NT, H, D + 1], BF16)  # [n_in, n_tile, head, d|1]
            attnT_t = []                            # one [c_in, n] tile per pair
            for hp in range(HP):
                attnT_t.append(atp.tile([P, N], BF16, tag=f"at{hp}",
                                        name=f"attnT_{hp}"))

            pctx = ctx.enter_context(ExitStack())
            xp = pctx.enter_context(tc.tile_pool(name="x", bufs=1))
            wap = pctx.enter_context(tc.tile_pool(name="wA", bufs=32))
            # io_ps only serves the front (warmup, v-projection slab 0, q/k
            # pair 0); it closes before the round-loop psum pools open so
            # the rounds get its banks back.
            frontctx = pctx.enter_context(ExitStack())
            io_ps = frontctx.enter_context(tc.tile_pool(name="io_ps", bufs=2, space="PSUM"))
            # ---- front DMAs ----
            # xT split across both DGE queues so all 8 slices land in ~5us;
            # q/k pair-0 weights (emitted by qk_proj_chunks(0) below) go to
            # the gpsimd queue behind the odd slices.  wv0 follows on sync
            # (needed by round 0's v-chains), wv1 later (round 1+).
            xT_sb = xp.tile([P, CT, N], BF16)
            hCT = CT // 2
            nc.sync.dma_start(
                xT_sb[:, 0:hCT, :],
                xTt.ap()[:, 0:hCT * N].rearrange("p (ct n) -> p ct n", n=N))

            # ---- constants ----
            # ones via DVE memset: no DMA dependency, so the warmup matmuls
            # below can start as soon as the engines boot (~2us).
            ones_bsq = consts.tile([P, P], BF16)    # all-ones square (bf16)
            nc.vector.memset(ones_bsq[:], 1.0)
            b_qk = consts.tile([P, 2 * CT], F32)    # q/k bias, per-partition
            nc.sync.dma_start(b_qk[:], b_qk_pm.ap())
            b_vb = consts.tile([P, C], BF16)        # v bias, partition-bcast
            nc.sync.dma_start(b_vb[:], b_v_bf.ap()[None, :].to_broadcast([P, C]))
            b_ob = consts.tile([P, C], BF16)        # out bias, partition-bcast
            nc.sync.dma_start(b_ob[:], b_o_bf.ap()[None, :].to_broadcast([P, C]))

            # ---- PE warmup ----
            # The HAM clock gate holds the PE at 1.2 GHz until it has seen
            # ~3.4us of sustained activity.  Spin matmuls on the ones tile
            # while the xT / weight DMAs are in flight so pair 0's q/k
            # chains start warm (2.4 GHz).  ~14 cold spins span ~6us,
            # bridging until the first data lands.
            wu_mov = consts.tile([P, IB], BF16)
            nc.vector.memset(wu_mov[:], 1.0)
            wu_ps = io_ps.tile([P, IB], F32, tag="iops", name="warmup")
            for _ in range(14):
                nc.tensor.matmul(wu_ps[:], ones_bsq[:], wu_mov[:],
                                 start=True, stop=True)

            # ones column of v_ext (free-dim broadcast copy from ones_bsq)
            nc.vector.tensor_copy(
                v_ext[:, :, :, D:D + 1],
                ones_bsq[:, None, None, 0:1].to_broadcast([P, NT, H, 1]))

            # ---- v projection, token-major ----
            # v[n, f'] = sum_c xT[c, n] * w_inT[c, 2C+f'] + b_in[2C+f']
            # Slab fb=0 (heads 0-7, first needed by pair 0's P@V in round 1)
            # fills round 0's empty P@V slot; slab fb=1 (heads 8-15, first
            # needed in round 5) spreads 2 chains per round over rounds 1-4.
            def v_chain(fb, nt, wvs, pool):
                hs = fb * (IB // D)
                he = (fb + 1) * (IB // D)
                ps = pool.tile([P, IB], F32, tag="iops", name=f"vps_{fb}_{nt}")
                for ct in range(CT):
                    nc.tensor.matmul(
                        ps[:], xT_sb[:, ct, nt * P:(nt + 1) * P],
                        wvs[ct], start=(ct == 0), stop=(ct == CT - 1))
                nc.vector.tensor_tensor(
                    v_ext[:, nt, hs:he, 0:D],
                    ps[:].rearrange("p (h d) -> p h d", d=D),
                    b_vb[:, fb * IB:(fb + 1) * IB].rearrange(
                        "p (h d) -> p h d", d=D),
                    mybir.AluOpType.add)

            wv0_t = wap.tile([P, CT, IB], BF16, tag="wv0", name="wv0",
                             bufs=1)
            nc.sync.dma_start(
                wv0_t[:], wv.ap()[0:P, :].rearrange(
                    "p (ct c) -> p ct c", c=IB))
            wv0 = [wv0_t[:, ct, :] for ct in range(CT)]


            # ---- q/k projection for one head pair ----
            # Emits the 16 weight-tile DMAs and 4 accumulation chains
            # (2 regions x 2 query blocks) of 8 matmuls each, as 8 chunks of
            # 4 matmuls for interleaving with attention.
            def qk_proj_chunks(hp):
                for reg in range(2):
                    qk_t[(reg, hp)] = qkp.tile([P, N], BF16, tag="qk",
                                               name=f"qk_{reg}_{hp}")
                dma = nc.gpsimd.dma_start if hp < 2 else nc.sync.dma_start
                wts_all = wap.tile([P, 2, CT, P], BF16, tag="w", bufs=3,
                                   name=f"wqk_{hp}")
                for reg in range(2):
                    r0 = (hp * 2 + reg) * P
                    dma(wts_all[:, reg],
                        wqk.ap()[r0:r0 + P, :].rearrange(
                            "p (ct c) -> p ct c", c=P))
                wts = {(reg, ct): wts_all[:, reg, ct, :]
                       for reg in range(2) for ct in range(CT)}
                chains = []
                for reg in range(2):
                    for nb in range(NIB):
                        chains.append((reg, nb))

                def chunk(i):                    # i in 0..7 -> half-chain
                    reg, nb = chains[i // 2]
                    ft = reg * CT + hp
                    ps_key = (reg, nb)
                    if i % 2 == 0:
                        qk_proj_chunks.ps[ps_key] = qk_proj_chunks.pool.tile(
                            [P, IB], F32, tag="iops", name=f"qkps_{hp}_{reg}_{nb}")
                    ps = qk_proj_chunks.ps[ps_key]
                    for ct in range(4 * (i % 2), 4 * (i % 2) + 4):
                        nc.tensor.matmul(
                            ps[:], wts[(reg, ct)],
                            xT_sb[:, ct, nb * IB:(nb + 1) * IB],
                            start=(ct == 0), stop=(ct == CT - 1))
                    if i % 2 == 1:
                        nc.vector.tensor_scalar_add(
                            qk_t[(reg, hp)][:, nb * IB:(nb + 1) * IB], ps[:],
                            b_qk[:, ft:ft + 1])
                return chunk
            qk_proj_chunks.ps = {}
            qk_proj_chunks.pool = io_ps

            # ---- attention ----
            # Per head pair hp the round interleaves, per key-tile step jt:
            #   2 score tiles (one per query block, both heads row-packed),
            #   their exp evictions (scalar engine, the round clock),
            #   one q/k projection chunk of pair hp+2,
            #   one P@V chunk of pair hp-1.
            # The score psum is double-buffered as two [128, 1024] tiles so
            # the exp of step k overlaps the score matmuls of step k+1 —
            # the baseline's single [128, 2048] buffer serialized
            # exp(k) -> scores(k+1) at ~2.9us per step.
            normb_q = []

            def flush_b():
                while normb_q:
                    normb_q.pop(0)()

            def norm_chain(hp, ib, pv0, pv1, defer=True):
                """Softmax normalization for one (pair, query-block).

                Four fused DMA hops (one descriptor each for spread /
                spread-back / broadcast, covering both heads): denominator
                rows -> DRAM -> spread [64,16] -> reciprocal (DVE) ->
                spread-back -> partition-broadcast [64,2,512].  The final
                mults write attnT (head 1 straight to partitions 64-127 —
                DVE operand base partitions need not match).  The three
                chains straddling the round->tail boundary each get their
                own DGE queue so they never convoy.
                """
                isl = slice(ib * IB, (ib + 1) * IB)
                last = hp == HP - 1
                if last:
                    q = nc.sync if ib == 0 else nc.gpsimd
                elif hp == HP - 2 and ib == 1:
                    q = nc.scalar
                else:
                    q = nc.gpsimd if (2 * hp + ib) % 2 == 0 else nc.sync
                pvc0 = pvcp.tile([D + 1, IB], F32, tag="pvc")
                sd2 = rdp.tile([2, IB], F32, tag="rd")
                if last:
                    # tail chains: stage both denominator rows in one tile
                    # (partitions 0/64 -> one sd descriptor); the mults read
                    # the P@V psum directly (those banks aren't recycled).
                    nc.vector.tensor_copy(pvc0[0:1, :], pv0[D:D + 1, :])
                    nc.vector.tensor_copy(pvc0[D:D + 1, :], pv1[D:D + 1, :])
                    q.dma_start(sd2[:], pvc0[0:D + 1:D, :])
                else:
                    # rounds: evict P@V to SBUF right away so the psum slots
                    # recycle for the next accumulation
                    pvc1 = pvcp.tile([D + 1, IB], F32, tag="pvc")
                    nc.vector.tensor_copy(pvc0[:], pv0[:])
                    nc.vector.tensor_copy(pvc1[:], pv1[:])
                    q.dma_start(sd2[0:1, :], pvc0[D:D + 1, :])
                    q.dma_start(sd2[1:2, :], pvc1[D:D + 1, :])
                    m0, m1 = pvc0, pvc1
                rsp = r2p.tile([D, 2 * (IB // D)], F32, tag="rsp")
                q.dma_start(
                    rsp[:].rearrange("p (h o) -> p h o", h=2),
                    sd2[:].rearrange("h (p o) -> p h o", p=D))
                nc.vector.reciprocal(rsp[:], rsp[:])
                rd2 = rdp.tile([2, IB], F32, tag="rd")
                q.dma_start(
                    rd2[:].rearrange("h (p o) -> p h o", p=D),
                    rsp[:].rearrange("p (h o) -> p h o", h=2))
                rb2 = rbp.tile([D, 2, IB], F32, tag="rb2", bufs=3)
                q.dma_start(rb2[:], rd2[:][None, :, :].to_broadcast(
                    [D, 2, IB]))

                def phase_b():
                    # normalized head outputs -> attnT.  Deferred until the
                    # rb broadcast has landed: the vector engine is strict
                    # FIFO, so a waiting mult would head-of-line block the
                    # psum-freeing copies of the next chain.
                    nc.vector.tensor_tensor(
                        attnT_t[hp][0:D, isl],
                        (pv0 if last else pvc0)[0:D, :], rb2[:, 0, :],
                        mybir.AluOpType.mult)
                    nc.vector.tensor_tensor(
                        attnT_t[hp][D:P, isl],
                        (pv1 if last else pvc1)[0:D, :], rb2[:, 1, :],
                        mybir.AluOpType.mult)
                if defer:
                    normb_q.append(phase_b)
                return phase_b

            def sc_exp(hp, jt, ib):
                # one [128, 1024] score tile: both heads for one query
                # block, written by a row-packed (concurrent) matmul pair,
                # evicted through exp in a single ACTIVATE.
                stt = st_ps.tile([P, N], F32, tag="st",
                                 name=f"st_{hp}_{jt}_{ib}")
                for h in (0, 1):
                    hsl = slice(h * D, (h + 1) * D)
                    nc.tensor.matmul(
                        stt[:, h * IB:(h + 1) * IB],
                        qk_t[(1, hp)][hsl, jt * P:(jt + 1) * P],
                        qk_t[(0, hp)][hsl, ib * IB:(ib + 1) * IB],
                        start=True, stop=True)
                pt_t = ptp.tile([P, N], BF16, tag="pt")
                nc.scalar.activation(pt_t[:], stt[:], EXP, scale=SCALE)
                return pt_t       # [:, h*IB:(h+1)*IB] = P^T of head h

            def pv_chunk(hp, jt, pts, pv0, pv1):
                # two key-tile steps of both heads' P@V accumulation chains
                # for query block jt//4 of pair hp.
                ib = jt // 4
                j2 = 2 * (jt % 4)
                for jj in (j2, j2 + 1):
                    fl = dict(start=(jj == 0), stop=(jj == NT - 1))
                    ptt = pts[(jj, ib)]
                    nc.tensor.matmul(
                        pv0[:], v_ext[:, jj, 2 * hp, :], ptt[:, 0:IB], **fl)
                    nc.tensor.matmul(
                        pv1[:], v_ext[:, jj, 2 * hp + 1, :],
                        ptt[:, IB:N], **fl)

            # software pipeline, two levels deep:
            #  - the q/k projection of pair hp+1 is interleaved into round hp
            #  - P@V of pair hp-1 runs during the score/exp stage of pair hp
            chunk0 = qk_proj_chunks(0)
            nc.gpsimd.dma_start(
                xT_sb[:, hCT:CT, :],
                xTt.ap()[:, hCT * N:].rearrange("p (ct n) -> p ct n", n=N))
            # wv1 after pair-0's weights so those win the gpsimd queue
            wv1_t = wap.tile([P, CT, IB], BF16, tag="wv1", name="wv1",
                             bufs=1)
            nc.gpsimd.dma_start(
                wv1_t[:], wv.ap()[P:2 * P, :].rearrange(
                    "p (ct c) -> p ct c", c=IB))
            wv1 = [wv1_t[:, ct, :] for ct in range(CT)]
            for i in range(8):
                chunk0(i)
            # front done: retire io_ps so the round pools get its banks.
            # Rounds psum budget: qk 1 + P@V 3 + scores 2x2 = 8 banks.
            frontctx.close()
            ptp = pctx.enter_context(tc.tile_pool(name="pt", bufs=28))
            qk_ps = pctx.enter_context(tc.tile_pool(name="qk_ps", bufs=2, space="PSUM"))
            pv_ps = pctx.enter_context(tc.tile_pool(name="pv_ps", bufs=2, space="PSUM"))
            stctx = pctx.enter_context(ExitStack())
            st_ps = stctx.enter_context(tc.tile_pool(name="st_ps", bufs=2, space="PSUM"))
            qk_proj_chunks.pool = qk_ps

            pending = qk_proj_chunks(1)
            pts_prev = None
            wo_all = {}

            # ---- out-projection helpers ----
            # y[n, c'] = sum_c attnT[c, n] * w_outT[c, c'] + b_out[c']
            # The k-step (ct) index coincides with the head-pair index, so a
            # chain over a ct subset only depends on those pairs' attnT.
            def oproj_chain(cb, nt, cts, pool, tag):
                ps = pool.tile([P, IB], F32, tag=tag,
                               name=f"ops_{cb}_{nt}_{cts[0]}")
                for k, ct in enumerate(cts):
                    nc.tensor.matmul(
                        ps[:], attnT_t[ct][:, nt * P:(nt + 1) * P],
                        wo_all[(cb, ct)], start=(k == 0),
                        stop=(k == len(cts) - 1))
                return ps

            for r in range(HP):
                nxt = qk_proj_chunks(r + 2) if r + 2 < HP else None
                if r == 5:
                    # out-projection weight prefetch: after pair 7's q/k
                    # weight DMAs (round-critical), early enough to land
                    # well before the tail consumes them.
                    for cb in range(C // IB):
                        wt = wbp.tile([P, CT, IB], BF16, tag="wo",
                                      name=f"wo_{cb}", bufs=2)
                        nc.sync.dma_start(
                            wt[:], wo.ap()[cb * P:(cb + 1) * P, :].rearrange(
                                "p (ct c) -> p ct c", c=IB))
                        for ct in range(CT):
                            wo_all[(cb, ct)] = wt[:, ct, :]
                chunks = pending
                ci = 0
                pts_cur = {}
                pv0 = pv1 = None
                fillers = []

                for jt in range(NT):
                    # step order sc, sc, pv, qk: the P@V chunk right after
                    # the scores puts its psum-freeing copies at the head of
                    # the vector queue, so the next P@V allocation never
                    # stalls the scores behind it.
                    pts_cur[(jt, 0)] = sc_exp(r, jt, 0)
                    pts_cur[(jt, 1)] = sc_exp(r, jt, 1)
                    if jt % 4 == 1:
                        # norm mults, 2 steps after their chain launch: the
                        # rb broadcasts have landed (batched weight DMAs
                        # keep the queues shallow), and flushing before the
                        # step-2 scores keeps the mults clear of the next
                        # pv-block allocation at jt%4==0.
                        flush_b()
                    if pts_prev is not None:
                        if jt % 4 == 0:
                            ib = jt // 4
                            pv0 = pv_ps.tile([D + 1, IB], F32, tag="pv",
                                             name=f"pv0_{r-1}_{ib}")
                            pv1 = pv_ps.tile([D + 1, IB], F32, tag="pv",
                                             name=f"pv1_{r-1}_{ib}")
                        pv_chunk(r - 1, jt, pts_prev, pv0, pv1)
                        if jt % 4 == 3:
                            norm_chain(r - 1, jt // 4, pv0, pv1)
                        if 1 <= r <= 4 and jt in (2, 6):
                            # second v slab (heads 8-15, first needed round
                            # 5): two chains per round out of the qk bank
                            v_chain(1, 2 * (r - 1) + (jt == 6), wv1, qk_ps)
                    else:
                        # round 0 has no P@V yet: the first v slab (heads
                        # 0-7, needed by round 1) runs in its slot, sharing
                        # the qk bank with pair 1's projection chunks.
                        v_chain(0, jt, wv0, qk_ps)
                    if chunks is not None and ci < 8:
                        chunks(ci)
                        ci += 1
                    elif fillers and jt % 2 == 1:
                        fillers.pop(0)()
                # flush any chunks the interleave did not reach
                if chunks is not None:
                    while ci < 8:
                        chunks(ci)
                        ci += 1
                while fillers:
                    fillers.pop(0)()
                pending = nxt
                pts_prev = pts_cur

            # ---- tail ----
            # Pair-7 P@V runs first (it gates everything), with a
            # short-latency norm chain: approx-reciprocal straight off the
            # PSUM denominator row, then one DRAM hop + one broadcast hop.
            # The remaining out-projection chains keep the PE dense while
            # those chains drain; the 16 pair-7 k-steps then close each
            # otile, half as soon as ib0's normalization lands.
            stctx.close()
            op_ps = pctx.enter_context(tc.tile_pool(name="op_ps", bufs=4, space="PSUM"))
            hp7 = HP - 1
            mults_q = []

            for ib in range(2):
                pva = pv_ps.tile([D + 1, IB], F32, tag="pv",
                                 name=f"pv0_{hp7}_{ib}")
                pvb = pv_ps.tile([D + 1, IB], F32, tag="pv",
                                 name=f"pv1_{hp7}_{ib}")
                for jj in range(NT):
                    fl = dict(start=(jj == 0), stop=(jj == NT - 1))
                    ptt = pts_prev[(jj, ib)]
                    nc.tensor.matmul(pva[:], v_ext[:, jj, 2 * hp7, :],
                                     ptt[:, 0:IB], **fl)
                    nc.tensor.matmul(pvb[:], v_ext[:, jj, 2 * hp7 + 1, :],
                                     ptt[:, IB:N], **fl)
                with tc.high_priority(offset=600):
                    mults_q.append(
                        norm_chain(hp7, ib, pva, pvb, defer=False))

            flush_b()          # pair-6 ib1 norm mults (rbs landed by now)
            mults_q.pop(0)()   # pair-7 ib0 mults (attnT_7 cols 0..511)

            # Split out-projection: ct0..6 partials for all 16 otiles keep
            # the PE dense for ~24us while pair-7's normalization chains
            # drain; the 16 pair-7 k-steps then run as one dense block with
            # the final add + store trailing on vector and the DGE queues.
            store_q = [nc.sync, nc.gpsimd, nc.scalar]
            y06 = {}
            kq = 0

            def oproj_partials(nts, flushk):
                for k, (nt, cb) in enumerate(
                        [(nt, cb) for nt in nts for cb in range(2)]):
                    ps = oproj_chain(cb, nt, list(range(CT - 1)), op_ps,
                                     "ops")
                    yt = yp.tile([P, IB], BF16, tag="y", bufs=8,
                                 name=f"y06_{cb}_{nt}")
                    nc.vector.tensor_tensor(
                        yt[:], ps[:], b_ob[:, cb * IB:(cb + 1) * IB],
                        mybir.AluOpType.add)
                    y06[(cb, nt)] = yt
                    if k == flushk:
                        mults_q.pop(0)()

            def oproj_ct7(nts):
                nonlocal kq
                for nt in nts:
                    for cb in range(2):
                        ps = oproj_chain(cb, nt, [CT - 1], op_ps, "ops")
                        yo = yp.tile([P, IB], F32, tag="yo", bufs=4)
                        nc.vector.tensor_tensor(
                            yo[:], ps[:], y06.pop((cb, nt))[:],
                            mybir.AluOpType.add)
                        store_q[kq % 3].dma_start(
                            y.ap()[nt * P:(nt + 1) * P,
                                   cb * IB:(cb + 1) * IB], yo[:])
                        kq += 1

            # first half: token tiles 0-3 (their pair-7 k-step only needs
            # ib0's normalization); second half overlaps the finishes
            oproj_partials((0, 1, 2, 3), 3)
            oproj_ct7((0, 1, 2, 3))
            oproj_partials((4, 5, 6, 7), -1)
            oproj_ct7((4, 5, 6, 7))

    nc.compile()
    return nc


def _get_nc():
    if "nc" not in _CACHE:
        _CACHE["nc"] = _build()
    return _CACHE["nc"]


def kernel(x, w_in, b_in, w_out, b_out):
    global LAST_EXEC_TIME_NS
    import ml_dtypes
    from concourse.bass_utils import run_bass_kernel_spmd

    bf16 = ml_dtypes.bfloat16
    x = np.asarray(x, dtype=np.float32)
    w_in = np.asarray(w_in, dtype=np.float32)
    b_in = np.asarray(b_in, dtype=np.float32)
    w_out = np.asarray(w_out, dtype=np.float32)
    b_out = np.asarray(b_out, dtype=np.float32)

    w_inT = np.ascontiguousarray(w_in.T).astype(bf16)
    w_outT = np.ascontiguousarray(w_out.T).astype(bf16)
    # pre-tiled weights: every device load becomes one fully-contiguous
    # DMA (partition-major [p, ct, c] order per tile)
    wqk = np.ascontiguousarray(
        w_inT[:, 0:2 * C].reshape(CT, P, 2, HP, P).transpose(3, 2, 1, 0, 4)
    ).reshape(HP * 2 * P, CT * P)
    wv = np.ascontiguousarray(
        w_inT[:, 2 * C:].reshape(CT, P, 2, IB).transpose(2, 1, 0, 3)
    ).reshape(2 * P, CT * IB)
    wo = np.ascontiguousarray(
        w_outT.reshape(CT, P, 2, IB).transpose(2, 1, 0, 3)
    ).reshape(2 * P, CT * IB)
    b_qk_pm = np.ascontiguousarray(b_in[0:2 * C].reshape(2 * CT, P).T)
    b_v = b_in[2 * C:F3].astype(bf16)
    b_o = b_out.astype(bf16)

    in_maps = []
    for b in range(B):
        xTt = np.ascontiguousarray(
            x[b].T.reshape(CT, P, N).transpose(1, 0, 2)
        ).reshape(P, CT * N).astype(bf16)
        in_maps.append({
            "xTt": xTt,
            "wqk": wqk,
            "wv": wv,
            "wo": wo,
            "b_qk_pm": b_qk_pm,
            "b_v_bf": b_v,
            "b_o_bf": b_o,
        })

    nc = _get_nc()

    trace = False
    tmpdir = None
    import os
    if os.environ.get("BASS_KERNEL_TRACE") == "1":
        trace = True
        try:
            import profshim  # noqa: F401
            import tempfile
            tmpdir = tempfile.mkdtemp(dir=profshim.TRACE_DIR)
        except Exception:
            tmpdir = None

    res = run_bass_kernel_spmd(nc, in_maps, core_ids=list(range(B)),
                               trace=trace, tmpdir=tmpdir)
    _CACHE["last_res"] = res
    LAST_EXEC_TIME_NS = res.exec_time_ns
    out = np.stack([res.results[b]["y"] for b in range(B)], axis=0)
    return out



# revision 44
# speedup vs baseline: 1.1834x; 1.1834x over previous
"""Trainium2 Bass kernel for multi-head self-attention (dense transformer block).

Reference computation (per batch element b):
    qkv  = x @ w_in.T + b_in                      # [N, 3C]
    q,k,v per head (H=16, D=64)
    S    = (q @ k.T) * D**-0.5
    P    = softmax(S)                             # rows over keys
    attn = P @ v                                  # [N, C] after head merge
    y    = attn @ w_out.T + b_out

Sharding: data-parallel over batch. B=8 maps one batch element per NeuronCore.

Per-core dataflow (bf16 matmuls, fp32 PSUM accumulation; the softmax
normalization chain runs in fp32r for accuracy):
  - host pre-transposes x[b] -> xT [C, N] and the weights (w_inT [C,3C],
    w_outT [C,C]) and casts them to bf16, so every matmul operand loads with
    a contiguous DMA at full PE rate (1 elem/cycle, FWL weight loads).
  - in_proj produces qT,kT in feature-major layout [f, n] (f on partitions,
    bias fused into the PSUM eviction as a per-partition DVE add) and v in
    token-major layout [n, f] (bias via a K=1 ones-matmul into the PSUM
    accumulation).  v is stored per head with a 65th all-ones column.
  - attention computes S^T per head ([keys, queries] layout) so that
    P^T = exp(S^T * scale) comes out with keys on partitions, ready to be the
    moving operand of the P@V matmul (contraction over keys).  Softmax skips
    the max-subtraction (scores are O(5) so exp is safe in fp32), which lets
    exp fuse into the PSUM->SBUF eviction on the scalar engine as one
    [128, 1024] op per (head, key-tile).  The P@V matmul uses the [v | ones]
    stationary (M=65): row 64 of the output is the softmax denominator,
    computed for free on the tensor engine.  Because the denominator sums the
    same bf16-rounded P used by P@V, the normalization is exact with respect
    to the rounding.
  - normalization: 1/rowsum on DVE (fp32r), broadcast across partitions via a
    K=1 fp32r ones-matmul (fp32r matmuls must write PSUM at partition 0, so
    head 1's normalized tile takes an SBUF->SBUF DMA hop to partitions
    64-127).
  - out_proj consumes the head-merged attnT [c, n] directly as the stationary
    operand; bias again via a K=1 ones-matmul.
  - the q/k projection matmuls of head pair hp+1 are interleaved into the
    attention of pair hp so the tensor engine stays saturated while the
    scalar engine drains the exp evictions (keeps the HAM clock gate at 8/8).

Optimizations over the original pipeline (308us -> ~266us):
  - PE warmup spin at kernel start (memset-built ones, no DMA dependency)
    so the HAM clock gate is at 8/8 before the v projection streams.
  - score psum double-buffered as 2x[128,1024] tiles: breaks the
    exp(k) -> scores(k+1) serialization that clocked the old rounds at
    ~2.9us/step; rounds now run at the ACTIVATE floor (~2.2us/step).
  - per key-tile step the round interleaves scores, P@V of the previous
    pair and a q/k chunk of the next pair, so the PE tracks the exp floor.
  - the second v-projection slab (heads 8-15, first needed in round 5) is
    deferred into round 0's empty P@V slot, hiding ~14us of front time.
  - softmax normalization split in two phases: the psum-freeing copies +
    reciprocal spread/broadcast DMAs launch at P@V stop, but the final
    multiplies are emitted ~2 steps later when their broadcasts have
    landed — the vector engine is strict FIFO, and a waiting mult would
    head-of-line block the copies that recycle the P@V psum banks.
  - split-y out projection: all waves accumulate head pairs 0..6 and bank
    the partial in SBUF (bf16); the pair-7 k-steps trail 3 waves behind,
    giving pair 7's normalization chains a ~15us runway before anything
    waits on them.
"""
import numpy as np
from contextlib import ExitStack

import concourse.bass as bass  # noqa: F401
from concourse import bacc
import concourse.tile as tile
from concourse import mybir

F32 = mybir.dt.float32
F32R = mybir.dt.float32r
BF16 = mybir.dt.bfloat16
EXP = mybir.ActivationFunctionType.Exp

B = 8
N = 1024          # tokens
C = 1024          # hidden
H = 16            # heads
D = C // H        # 64
F3 = 3 * C
SCALE = float(D) ** -0.5
P = 128
CT = C // P       # 8 contraction tiles over C
NT = N // P       # 8 token tiles
HP = H // 2       # 8 head pairs (two heads share a 128-partition tile)
IB = 512          # query-block (matmul moving free dim)
NIB = N // IB     # 2

_CACHE = {}
LAST_EXEC_TIME_NS = None


def _build():
    nc = bacc.Bacc("TRN2", target_bir_lowering=False, debug=False)
    xTt = nc.dram_tensor("xTt", [P, CT * N], BF16, kind="ExternalInput")
    wqk = nc.dram_tensor("wqk", [HP * 2 * P, CT * P], BF16,
                         kind="ExternalInput")
    wv = nc.dram_tensor("wv", [2 * P, CT * IB], BF16, kind="ExternalInput")
    wo = nc.dram_tensor("wo", [2 * P, CT * IB], BF16, kind="ExternalInput")
    b_qk_pm = nc.dram_tensor("b_qk_pm", [P, 2 * CT], F32, kind="ExternalInput")
    b_v_bf = nc.dram_tensor("b_v_bf", [C], BF16, kind="ExternalInput")
    b_o_bf = nc.dram_tensor("b_o_bf", [C], BF16, kind="ExternalInput")
    y = nc.dram_tensor("y", [N, C], F32, kind="ExternalOutput")

    with tile.TileContext(nc) as tc:
        with ExitStack() as ctx:
            consts = ctx.enter_context(tc.tile_pool(name="consts", bufs=1))
            qkp = ctx.enter_context(tc.tile_pool(name="qk", bufs=7))
            vp = ctx.enter_context(tc.tile_pool(name="v", bufs=1))
            atp = ctx.enter_context(tc.tile_pool(name="attnT", bufs=1))
            wbp = ctx.enter_context(tc.tile_pool(name="wB", bufs=16))
            yp = ctx.enter_context(tc.tile_pool(name="y", bufs=3))
            r2p = ctx.enter_context(tc.tile_pool(name="r2", bufs=4))
            pvcp = ctx.enter_context(tc.tile_pool(name="pvc", bufs=6))
            rbp = ctx.enter_context(tc.tile_pool(name="rb", bufs=6))
            rdp = ctx.enter_context(tc.tile_pool(name="rd", bufs=16, space="DRAM"))

            # ---- persistent big tensors ----
            # q/k in feature-major layout, one tile per (region, head pair) so
            # attention of pair hp depends only on its own projection tiles.
            qk_t = {}                           # allocated lazily, 8 slots rotate
            v_ext = vp.tile([P, NT, H, D + 1], BF16)  # [n_in, n_tile, head, d|1]
            attnT_t = []                            # one [c_in, n] tile per pair
            for hp in range(HP):
                attnT_t.append(atp.tile([P, N], BF16, tag=f"at{hp}",
                                        name=f"attnT_{hp}"))

            pctx = ctx.enter_context(ExitStack())
            xp = pctx.enter_context(tc.tile_pool(name="x", bufs=1))
            wap = pctx.enter_context(tc.tile_pool(name="wA", bufs=32))
            # io_ps only serves the front (warmup, v-projection slab 0, q/k
            # pair 0); it closes before the round-loop psum pools open so
            # the rounds get its banks back.
            frontctx = pctx.enter_context(ExitStack())
            io_ps = frontctx.enter_context(tc.tile_pool(name="io_ps", bufs=2, space="PSUM"))
            # ---- front DMAs ----
            # xT split across both DGE queues so all 8 slices land in ~5us;
            # q/k pair-0 weights (emitted by qk_proj_chunks(0) below) go to
            # the gpsimd queue behind the odd slices.  wv0 follows on sync
            # (needed by round 0's v-chains), wv1 later (round 1+).
            xT_sb = xp.tile([P, CT, N], BF16)
            hCT = CT // 2
            nc.sync.dma_start(
                xT_sb[:, 0:hCT, :],
                xTt.ap()[:, 0:hCT * N].rearrange("p (ct n) -> p ct n", n=N))

            # ---- constants ----
            # ones via DVE memset: no DMA dependency, so the warmup matmuls
            # below can start as soon as the engines boot (~2us).
            ones_bsq = consts.tile([P, P], BF16)    # all-ones square (bf16)
            nc.vector.memset(ones_bsq[:], 1.0)
            b_qk = consts.tile([P, 2 * CT], F32)    # q/k bias, per-partition
            nc.sync.dma_start(b_qk[:], b_qk_pm.ap())
            b_vb = consts.tile([P, C], BF16)        # v bias, partition-bcast
            nc.sync.dma_start(b_vb[:], b_v_bf.ap()[None, :].to_broadcast([P, C]))
            b_ob = consts.tile([P, C], BF16)        # out bias, partition-bcast
            nc.sync.dma_start(b_ob[:], b_o_bf.ap()[None, :].to_broadcast([P, C]))

            # ---- PE warmup ----
            # The HAM clock gate holds the PE at 1.2 GHz until it has seen
            # ~3.4us of sustained activity.  Spin matmuls on the ones tile
            # while the xT / weight DMAs are in flight so pair 0's q/k
            # chains start warm (2.4 GHz).  ~14 cold spins span ~6us,
            # bridging until the first data lands.
            wu_mov = consts.tile([P, IB], BF16)
            nc.vector.memset(wu_mov[:], 1.0)
            wu_ps = io_ps.tile([P, IB], F32, tag="iops", name="warmup")
            for _ in range(26):
                nc.tensor.matmul(wu_ps[:], ones_bsq[:], wu_mov[:],
                                 start=True, stop=True)

            # ones column of v_ext (free-dim broadcast copy from ones_bsq)
            nc.vector.tensor_copy(
                v_ext[:, :, :, D:D + 1],
                ones_bsq[:, None, None, 0:1].to_broadcast([P, NT, H, 1]))

            # ---- v projection, token-major ----
            # v[n, f'] = sum_c xT[c, n] * w_inT[c, 2C+f'] + b_in[2C+f']
            # Slab fb=0 (heads 0-7, first needed by pair 0's P@V in round 1)
            # fills round 0's empty P@V slot; slab fb=1 (heads 8-15, first
            # needed in round 5) spreads 2 chains per round over rounds 1-4.
            def v_chain(fb, nt, wvs, pool):
                hs = fb * (IB // D)
                he = (fb + 1) * (IB // D)
                ps = pool.tile([P, IB], F32, tag="iops", name=f"vps_{fb}_{nt}")
                for ct in range(CT):
                    nc.tensor.matmul(
                        ps[:], xT_sb[:, ct, nt * P:(nt + 1) * P],
                        wvs[ct], start=(ct == 0), stop=(ct == CT - 1))
                nc.vector.tensor_tensor(
                    v_ext[:, nt, hs:he, 0:D],
                    ps[:].rearrange("p (h d) -> p h d", d=D),
                    b_vb[:, fb * IB:(fb + 1) * IB].rearrange(
                        "p (h d) -> p h d", d=D),
                    mybir.AluOpType.add)

            wv0_t = wap.tile([P, CT, IB], BF16, tag="wv0", name="wv0",
                             bufs=1)
            nc.sync.dma_start(
                wv0_t[:], wv.ap()[0:P, :].rearrange(
                    "p (ct c) -> p ct c", c=IB))
            wv0 = [wv0_t[:, ct, :] for ct in range(CT)]


            # ---- q/k projection for one head pair ----
            # Emits the 16 weight-tile DMAs and 4 accumulation chains
            # (2 regions x 2 query blocks) of 8 matmuls each, as 8 chunks of
            # 4 matmuls for interleaving with attention.
            def qk_proj_chunks(hp):
                for reg in range(2):
                    qk_t[(reg, hp)] = qkp.tile([P, N], BF16, tag="qk",
                                               name=f"qk_{reg}_{hp}")
                dma = nc.gpsimd.dma_start if hp < 2 else nc.sync.dma_start
                wts_all = wap.tile([P, 2, CT, P], BF16, tag="w", bufs=4,
                                   name=f"wqk_{hp}")
                for reg in range(2):
                    r0 = (hp * 2 + reg) * P
                    dma(wts_all[:, reg],
                        wqk.ap()[r0:r0 + P, :].rearrange(
                            "p (ct c) -> p ct c", c=P))
                wts = {(reg, ct): wts_all[:, reg, ct, :]
                       for reg in range(2) for ct in range(CT)}
                chains = []
                for reg in range(2):
                    for nb in range(NIB):
                        chains.append((reg, nb))

                def chunk(i):                    # i in 0..7 -> half-chain
                    reg, nb = chains[i // 2]
                    ft = reg * CT + hp
                    ps_key = (reg, nb)
                    if i % 2 == 0:
                        qk_proj_chunks.ps[ps_key] = qk_proj_chunks.pool.tile(
                            [P, IB], F32, tag="iops", name=f"qkps_{hp}_{reg}_{nb}")
                    ps = qk_proj_chunks.ps[ps_key]
                    for ct in range(4 * (i % 2), 4 * (i % 2) + 4):
                        nc.tensor.matmul(
                            ps[:], wts[(reg, ct)],
                            xT_sb[:, ct, nb * IB:(nb + 1) * IB],
                            start=(ct == 0), stop=(ct == CT - 1))
                    if i % 2 == 1:
                        nc.vector.tensor_scalar_add(
                            qk_t[(reg, hp)][:, nb * IB:(nb + 1) * IB], ps[:],
                            b_qk[:, ft:ft + 1])
                return chunk
            qk_proj_chunks.ps = {}
            qk_proj_chunks.pool = io_ps

            # ---- attention ----
            # Per head pair hp the round interleaves, per key-tile step jt:
            #   2 score tiles (one per query block, both heads row-packed),
            #   their exp evictions (scalar engine, the round clock),
            #   one q/k projection chunk of pair hp+2,
            #   one P@V chunk of pair hp-1.
            # The score psum is double-buffered as two [128, 1024] tiles so
            # the exp of step k overlaps the score matmuls of step k+1 —
            # the baseline's single [128, 2048] buffer serialized
            # exp(k) -> scores(k+1) at ~2.9us per step.
            normb_q = []

            def flush_b():
                while normb_q:
                    normb_q.pop(0)()

            def norm_chain(hp, ib, pv0, pv1, defer=True):
                """Softmax normalization for one (pair, query-block).

                Four fused DMA hops (one descriptor each for spread /
                spread-back / broadcast, covering both heads): denominator
                rows -> DRAM -> spread [64,16] -> reciprocal (DVE) ->
                spread-back -> partition-broadcast [64,2,512].  The final
                mults write attnT (head 1 straight to partitions 64-127 —
                DVE operand base partitions need not match).  The three
                chains straddling the round->tail boundary each get their
                own DGE queue so they never convoy.
                """
                isl = slice(ib * IB, (ib + 1) * IB)
                last = hp == HP - 1
                if last:
                    q = nc.sync if ib == 0 else nc.gpsimd
                elif hp == HP - 2 and ib == 1:
                    q = nc.scalar
                else:
                    q = nc.gpsimd if (2 * hp + ib) % 2 == 0 else nc.sync
                pvc0 = pvcp.tile([D + 1, IB], F32, tag="pvc")
                sd2 = rdp.tile([2, IB], F32, tag="rd")
                if last:
                    # tail chains: stage both denominator rows in one tile
                    # (partitions 0/64 -> one sd descriptor); the mults read
                    # the P@V psum directly (those banks aren't recycled).
                    nc.vector.tensor_copy(pvc0[0:1, :], pv0[D:D + 1, :])
                    nc.vector.tensor_copy(pvc0[D:D + 1, :], pv1[D:D + 1, :])
                    q.dma_start(sd2[:], pvc0[0:D + 1:D, :])
                else:
                    # rounds: evict P@V to SBUF right away so the psum slots
                    # recycle for the next accumulation
                    pvc1 = pvcp.tile([D + 1, IB], F32, tag="pvc")
                    nc.vector.tensor_copy(pvc0[:], pv0[:])
                    nc.vector.tensor_copy(pvc1[:], pv1[:])
                    q.dma_start(sd2[0:1, :], pvc0[D:D + 1, :])
                    q.dma_start(sd2[1:2, :], pvc1[D:D + 1, :])
                    m0, m1 = pvc0, pvc1
                rsp = r2p.tile([D, 2 * (IB // D)], F32, tag="rsp")
                q.dma_start(
                    rsp[:].rearrange("p (h o) -> p h o", h=2),
                    sd2[:].rearrange("h (p o) -> p h o", p=D))
                nc.vector.reciprocal(rsp[:], rsp[:])
                rd2 = rdp.tile([2, IB], F32, tag="rd")
                q.dma_start(
                    rd2[:].rearrange("h (p o) -> p h o", p=D),
                    rsp[:].rearrange("p (h o) -> p h o", h=2))
                rb2 = rbp.tile([D, 2, IB], F32, tag="rb2", bufs=3)
                q.dma_start(rb2[:], rd2[:][None, :, :].to_broadcast(
                    [D, 2, IB]))

                def phase_b():
                    # normalized head outputs -> attnT.  Deferred until the
                    # rb broadcast has landed: the vector engine is strict
                    # FIFO, so a waiting mult would head-of-line block the
                    # psum-freeing copies of the next chain.
                    nc.vector.tensor_tensor(
                        attnT_t[hp][0:D, isl],
                        (pv0 if last else pvc0)[0:D, :], rb2[:, 0, :],
                        mybir.AluOpType.mult)
                    nc.vector.tensor_tensor(
                        attnT_t[hp][D:P, isl],
                        (pv1 if last else pvc1)[0:D, :], rb2[:, 1, :],
                        mybir.AluOpType.mult)
                if defer:
                    normb_q.append(phase_b)
                return phase_b

            def sc_exp(hp, jt, ib):
                # one [128, 1024] score tile: both heads for one query
                # block, written by a row-packed (concurrent) matmul pair,
                # evicted through exp in a single ACTIVATE.
                stt = st_ps.tile([P, N], F32, tag="st",
                                 name=f"st_{hp}_{jt}_{ib}")
                for h in (0, 1):
                    hsl = slice(h * D, (h + 1) * D)
                    nc.tensor.matmul(
                        stt[:, h * IB:(h + 1) * IB],
                        qk_t[(1, hp)][hsl, jt * P:(jt + 1) * P],
                        qk_t[(0, hp)][hsl, ib * IB:(ib + 1) * IB],
                        start=True, stop=True)
                pt_t = ptp.tile([P, N], BF16, tag="pt")
                nc.scalar.activation(pt_t[:], stt[:], EXP, scale=SCALE)
                return pt_t       # [:, h*IB:(h+1)*IB] = P^T of head h

            def pv_chunk(hp, jt, pts, pv0, pv1):
                # two key-tile steps of both heads' P@V accumulation chains
                # for query block jt//4 of pair hp.
                ib = jt // 4
                j2 = 2 * (jt % 4)
                for jj in (j2, j2 + 1):
                    fl = dict(start=(jj == 0), stop=(jj == NT - 1))
                    ptt = pts[(jj, ib)]
                    nc.tensor.matmul(
                        pv0[:], v_ext[:, jj, 2 * hp, :], ptt[:, 0:IB], **fl)
                    nc.tensor.matmul(
                        pv1[:], v_ext[:, jj, 2 * hp + 1, :],
                        ptt[:, IB:N], **fl)

            # software pipeline, two levels deep:
            #  - the q/k projection of pair hp+1 is interleaved into round hp
            #  - P@V of pair hp-1 runs during the score/exp stage of pair hp
            chunk0 = qk_proj_chunks(0)
            nc.gpsimd.dma_start(
                xT_sb[:, hCT:CT, :],
                xTt.ap()[:, hCT * N:].rearrange("p (ct n) -> p ct n", n=N))
            # wv1 after pair-0's weights so those win the gpsimd queue
            wv1_t = wap.tile([P, CT, IB], BF16, tag="wv1", name="wv1",
                             bufs=1)
            nc.gpsimd.dma_start(
                wv1_t[:], wv.ap()[P:2 * P, :].rearrange(
                    "p (ct c) -> p ct c", c=IB))
            wv1 = [wv1_t[:, ct, :] for ct in range(CT)]
            for i in range(8):
                chunk0(i)
            # front done: retire io_ps so the round pools get its banks.
            # Rounds psum budget: qk 1 + P@V 3 + scores 2x2 = 8 banks.
            frontctx.close()
            ptp = pctx.enter_context(tc.tile_pool(name="pt", bufs=26))
            qk_ps = pctx.enter_context(tc.tile_pool(name="qk_ps", bufs=2, space="PSUM"))
            pv_ps = pctx.enter_context(tc.tile_pool(name="pv_ps", bufs=2, space="PSUM"))
            stctx = pctx.enter_context(ExitStack())
            st_ps = stctx.enter_context(tc.tile_pool(name="st_ps", bufs=2, space="PSUM"))
            qk_proj_chunks.pool = qk_ps

            pending = qk_proj_chunks(1)
            pts_prev = None
            wo_all = {}

            # ---- out-projection helpers ----
            # y[n, c'] = sum_c attnT[c, n] * w_outT[c, c'] + b_out[c']
            # The k-step (ct) index coincides with the head-pair index, so a
            # chain over a ct subset only depends on those pairs' attnT.
            def oproj_chain(cb, nt, cts, pool, tag):
                ps = pool.tile([P, IB], F32, tag=tag,
                               name=f"ops_{cb}_{nt}_{cts[0]}")
                for k, ct in enumerate(cts):
                    nc.tensor.matmul(
                        ps[:], attnT_t[ct][:, nt * P:(nt + 1) * P],
                        wo_all[(cb, ct)], start=(k == 0),
                        stop=(k == len(cts) - 1))
                return ps

            for r in range(HP):
                nxt = qk_proj_chunks(r + 2) if r + 2 < HP else None
                if r == 5:
                    # out-projection weight prefetch: after pair 7's q/k
                    # weight DMAs (round-critical), early enough to land
                    # well before the tail consumes them.
                    for cb in range(C // IB):
                        wt = wbp.tile([P, CT, IB], BF16, tag="wo",
                                      name=f"wo_{cb}", bufs=2)
                        nc.sync.dma_start(
                            wt[:], wo.ap()[cb * P:(cb + 1) * P, :].rearrange(
                                "p (ct c) -> p ct c", c=IB))
                        for ct in range(CT):
                            wo_all[(cb, ct)] = wt[:, ct, :]
                chunks = pending
                ci = 0
                pts_cur = {}
                pv0 = pv1 = None
                fillers = []

                for jt in range(NT):
                    # step order sc, sc, pv, qk: the P@V chunk right after
                    # the scores puts its psum-freeing copies at the head of
                    # the vector queue, so the next P@V allocation never
                    # stalls the scores behind it.
                    pts_cur[(jt, 0)] = sc_exp(r, jt, 0)
                    pts_cur[(jt, 1)] = sc_exp(r, jt, 1)
                    if jt % 4 == 1:
                        # norm mults, 2 steps after their chain launch: the
                        # rb broadcasts have landed (batched weight DMAs
                        # keep the queues shallow), and flushing before the
                        # step-2 scores keeps the mults clear of the next
                        # pv-block allocation at jt%4==0.
                        flush_b()
                    if pts_prev is not None:
                        if jt % 4 == 0:
                            ib = jt // 4
                            pv0 = pv_ps.tile([D + 1, IB], F32, tag="pv",
                                             name=f"pv0_{r-1}_{ib}")
                            pv1 = pv_ps.tile([D + 1, IB], F32, tag="pv",
                                             name=f"pv1_{r-1}_{ib}")
                        pv_chunk(r - 1, jt, pts_prev, pv0, pv1)
                        if jt % 4 == 3:
                            norm_chain(r - 1, jt // 4, pv0, pv1)
                        if 1 <= r <= 4 and jt in (2, 6):
                            # second v slab (heads 8-15, first needed round
                            # 5): two chains per round out of the qk bank
                            v_chain(1, 2 * (r - 1) + (jt == 6), wv1, qk_ps)
                    else:
                        # round 0 has no P@V yet: the first v slab (heads
                        # 0-7, needed by round 1) runs in its slot, sharing
                        # the qk bank with pair 1's projection chunks.
                        v_chain(0, jt, wv0, qk_ps)
                    if chunks is not None and ci < 8:
                        chunks(ci)
                        ci += 1
                    elif fillers and jt % 2 == 1:
                        fillers.pop(0)()
                # flush any chunks the interleave did not reach
                if chunks is not None:
                    while ci < 8:
                        chunks(ci)
                        ci += 1
                while fillers:
                    fillers.pop(0)()
                pending = nxt
                pts_prev = pts_cur

            # ---- tail ----
            # Pair-7 P@V runs first (it gates everything), with a
            # short-latency norm chain: approx-reciprocal straight off the
            # PSUM denominator row, then one DRAM hop + one broadcast hop.
            # The remaining out-projection chains keep the PE dense while
            # those chains drain; the 16 pair-7 k-steps then close each
            # otile, half as soon as ib0's normalization lands.
            stctx.close()
            op_ps = pctx.enter_context(tc.tile_pool(name="op_ps", bufs=4, space="PSUM"))
            hp7 = HP - 1
            mults_q = []

            for ib in range(2):
                pva = pv_ps.tile([D + 1, IB], F32, tag="pv",
                                 name=f"pv0_{hp7}_{ib}")
                pvb = pv_ps.tile([D + 1, IB], F32, tag="pv",
                                 name=f"pv1_{hp7}_{ib}")
                for jj in range(NT):
                    fl = dict(start=(jj == 0), stop=(jj == NT - 1))
                    ptt = pts_prev[(jj, ib)]
                    nc.tensor.matmul(pva[:], v_ext[:, jj, 2 * hp7, :],
                                     ptt[:, 0:IB], **fl)
                    nc.tensor.matmul(pvb[:], v_ext[:, jj, 2 * hp7 + 1, :],
                                     ptt[:, IB:N], **fl)
                with tc.high_priority(offset=600):
                    mults_q.append(
                        norm_chain(hp7, ib, pva, pvb, defer=False))

            flush_b()          # pair-6 ib1 norm mults (rbs landed by now)
            mults_q.pop(0)()   # pair-7 ib0 mults (attnT_7 cols 0..511)

            # Split out-projection: ct0..6 partials for all 16 otiles keep
            # the PE dense for ~24us while pair-7's normalization chains
            # drain; the 16 pair-7 k-steps then run as one dense block with
            # the final add + store trailing on vector and the DGE queues.
            store_q = [nc.sync, nc.gpsimd, nc.scalar]
            y06 = {}
            kq = 0

            def oproj_partials(nts, flushk):
                for k, (nt, cb) in enumerate(
                        [(nt, cb) for nt in nts for cb in range(2)]):
                    ps = oproj_chain(cb, nt, list(range(CT - 1)), op_ps,
                                     "ops")
                    yt = yp.tile([P, IB], BF16, tag="y", bufs=8,
                                 name=f"y06_{cb}_{nt}")
                    nc.vector.tensor_tensor(
                        yt[:], ps[:], b_ob[:, cb * IB:(cb + 1) * IB],
                        mybir.AluOpType.add)
                    y06[(cb, nt)] = yt
                    if k == flushk:
                        mults_q.pop(0)()

            def oproj_ct7(nts):
                nonlocal kq
                for nt in nts:
                    for cb in range(2):
                        ps = oproj_chain(cb, nt, [CT - 1], op_ps, "ops")
                        yo = yp.tile([P, IB], F32, tag="yo", bufs=4)
                        nc.vector.tensor_tensor(
                            yo[:], ps[:], y06.pop((cb, nt))[:],
                            mybir.AluOpType.add)
                        store_q[kq % 3].dma_start(
                            y.ap()[nt * P:(nt + 1) * P,
                                   cb * IB:(cb + 1) * IB], yo[:])
                        kq += 1

            # first half: token tiles 0-3 (their pair-7 k-step only needs
            # ib0's normalization); second half overlaps the finishes
            oproj_partials((0, 1, 2, 3), 3)
            oproj_ct7((0, 1, 2, 3))
            oproj_partials((4, 5, 6, 7), -1)
            oproj_ct7((4, 5, 6, 7))

    nc.compile()
    return nc


def _get_nc():
    if "nc" not in _CACHE:
        _CACHE["nc"] = _build()
    return _CACHE["nc"]


def kernel(x, w_in, b_in, w_out, b_out):
    global LAST_EXEC_TIME_NS
    import ml_dtypes
    from concourse.bass_utils import run_bass_kernel_spmd

    bf16 = ml_dtypes.bfloat16
    x = np.asarray(x, dtype=np.float32)
    w_in = np.asarray(w_in, dtype=np.float32)
    b_in = np.asarray(b_in, dtype=np.float32)
    w_out = np.asarray(w_out, dtype=np.float32)
    b_out = np.asarray(b_out, dtype=np.float32)

    w_inT = np.ascontiguousarray(w_in.T).astype(bf16)
    w_outT = np.ascontiguousarray(w_out.T).astype(bf16)
    # pre-tiled weights: every device load becomes one fully-contiguous
    # DMA (partition-major [p, ct, c] order per tile)
    wqk = np.ascontiguousarray(
        w_inT[:, 0:2 * C].reshape(CT, P, 2, HP, P).transpose(3, 2, 1, 0, 4)
    ).reshape(HP * 2 * P, CT * P)
    wv = np.ascontiguousarray(
        w_inT[:, 2 * C:].reshape(CT, P, 2, IB).transpose(2, 1, 0, 3)
    ).reshape(2 * P, CT * IB)
    wo = np.ascontiguousarray(
        w_outT.reshape(CT, P, 2, IB).transpose(2, 1, 0, 3)
    ).reshape(2 * P, CT * IB)
    b_qk_pm = np.ascontiguousarray(b_in[0:2 * C].reshape(2 * CT, P).T)
    b_v = b_in[2 * C:F3].astype(bf16)
    b_o = b_out.astype(bf16)

    in_maps = []
    for b in range(B):
        xTt = np.ascontiguousarray(
            x[b].T.reshape(CT, P, N).transpose(1, 0, 2)
        ).reshape(P, CT * N).astype(bf16)
        in_maps.append({
            "xTt": xTt,
            "wqk": wqk,
            "wv": wv,
            "wo": wo,
            "b_qk_pm": b_qk_pm,
            "b_v_bf": b_v,
            "b_o_bf": b_o,
        })

    nc = _get_nc()

    trace = False
    tmpdir = None
    import os
    if os.environ.get("BASS_KERNEL_TRACE") == "1":
        trace = True
        try:
            import profshim  # noqa: F401
            import tempfile
            tmpdir = tempfile.mkdtemp(dir=profshim.TRACE_DIR)
        except Exception:
            tmpdir = None

    res = run_bass_kernel_spmd(nc, in_maps, core_ids=list(range(B)),
                               trace=trace, tmpdir=tmpdir)
    _CACHE["last_res"] = res
    LAST_EXEC_TIME_NS = res.exec_time_ns
    out = np.stack([res.results[b]["y"] for b in range(B)], axis=0)
    return out

